# revision 40
# baseline (speedup 1.0000x reference)
"""MADPSNet MoE-routing kernel for 8 Trainium2 NeuronCores.

The reference computes every expert on the full stacked input and then
gathers one expert per agent.  The routing indices (laac_shallow /
laac_deep) are host-visible numpy values, so we do the routing on the
host: per agent we select the 4 weight matrices of its chosen experts
and run only the selected chain

    x[2048,256] @ W1[256,512] -> relu -> @ W2[512,256] -> relu
                -> @ W3[256,512] -> relu -> @ W4[512,128] (+bias)

One agent per NeuronCore (A == 8 == n_cores), no collectives.

Layout: everything feature-major on chip (features on the 128
partitions, batch on the free dim).  The host pre-packs

    x   [128, 4096]     col = t*512 + k*256 + b  (t = 256-col L1 sub-tile)
    wN  [128, K/128*M]  col = (k*mc + m)*128 + j   (k-chunk-major)
    bias[128, 11]       col j = 128-chunk j of [b1(4) b2(2) b3(4) b4(1)]

Data is bf16 (PSUM accumulates fp32; rel err ~5e-3, well inside the
2e-2 gate) so every input transfer is half the bytes of f32 and
LDWEIGHTS runs under FWL.

The schedule is built around a measured constraint: a DMA's completion
sem fires ~1.5-2us after its data lands (HBM write-receipt round trip
under full 8-core load), so the earliest-possible first real matmul is
set by the size of the first transfer, not by bandwidth.  Layer 1
therefore runs on 256-col sub-tiles whose 128KB x chunks (each
carrying both k-halves, so the k-sweep never waits mid-accumulation)
sem ~1.5us earlier than a 256KB tile would; x goes on the sync HWDGE
queue, all weights on the scalar HWDGE queue in consumption order.
Warm-up matmuls on a scratch tile (only one column memset, just enough
to materialize it) bridge from engine start (~7us) to first-data
(~10us), opening the HAM clock window so the stream runs at 2.4 GHz,
and filler scratch matmuls between the first layer-1 sub-tiles absorb
x-sem arrival jitter so a late chunk can't idle the PE long enough to
re-throttle the clock; layers are emitted as a (bt + 2*layer) diagonal
wavefront so the in-order PE queue always has ready work (measured PE
gaps < 1us total).  Post-matmul relu is split across ScalarE and VectorE with a
fixed engine per destination tile; the final layer's PSUM->SBUF copies
split across both engines, its out-DMAs alternate queues, and the last
batch-tile runs as two 256-col accumulations so the drain is one
half-size transfer + receipt deep.  The kernel returns out^T
[128, 2048] per core; the host transposes back.
"""

import os

import numpy as np

import concourse.bass as bass
import concourse.mybir as mybir
from concourse import bacc
from concourse.bass_utils import run_bass_kernel_spmd
from concourse.tile import TileContext

A, B, S = 8, 2048, 256
H1, H2, D1, D2 = 512, 256, 512, 128
P = 128
BT = 512            # batch tile (psum bank: 512 fp32) for layers 2-4
NBT = B // BT
BT1 = 256           # layer-1 sub-tile: its 128KB x chunk (both k-halves)
NT1 = B // BT1      # sems ~1.5us earlier than a 256KB transfer would

_DT_MAP = {
    "f32": mybir.dt.float32,
    "f32r": mybir.dt.float32r,
    "bf16": mybir.dt.bfloat16,
}

# layer: (k_chunks, m_chunks, bias col offset, relu?)
_LAYERS = [
    (S // P, H1 // P, 0, True),    # L1: 256 -> 512
    (H1 // P, H2 // P, 4, True),   # L2: 512 -> 256
    (H2 // P, D1 // P, 6, True),   # L3: 256 -> 512
    (D1 // P, D2 // P, 10, False), # L4: 512 -> 128
]


def _build(dt_name: str, add_bias: bool, warm: int) -> bass.Bass:
    dt = _DT_MAP[dt_name]
    f32 = mybir.dt.float32
    nc = bacc.Bacc(None, target_bir_lowering=False, debug=False)

    x_d = nc.dram_tensor("x", [P, (S // P) * B], dt, kind="ExternalInput")
    w_ds = [
        nc.dram_tensor("w1", [P, (S // P) * H1], dt, kind="ExternalInput"),
        nc.dram_tensor("w2", [P, (H1 // P) * H2], dt, kind="ExternalInput"),
        nc.dram_tensor("w3", [P, (H2 // P) * D1], dt, kind="ExternalInput"),
        nc.dram_tensor("w4", [P, (D1 // P) * D2], dt, kind="ExternalInput"),
    ]
    b_d = (
        nc.dram_tensor("bias", [P, 11], f32, kind="ExternalInput")
        if add_bias
        else None
    )
    # bf16 mode also stages + ships the output in bf16 (half the final
    # drain bytes; the host upcasts, adding ~2e-3 rel err — far inside
    # the 2e-2 gate)
    odt = dt if dt == mybir.dt.bfloat16 else f32
    out_d = nc.dram_tensor("out", [D2, B], odt, kind="ExternalOutput")

    with TileContext(nc) as tc:
        with (
            tc.tile_pool(name="persist", bufs=1) as pp,
            tc.tile_pool(name="psum", bufs=8, space="PSUM") as psp,
        ):
            xt = pp.tile([P, (S // P) * B], dt, tag="xt", name="xt")
            wts = [
                pp.tile(
                    [P, w_ds[i].shape[1]], dt, tag=f"w{i}", name=f"w{i}_sb"
                )
                for i in range(4)
            ]
            bti = (
                pp.tile([P, 11], f32, tag="bias", name="bias_sb")
                if add_bias
                else None
            )
            scr = (
                pp.tile([P, 2], f32, tag="scr", name="scr") if add_bias else None
            )
            acts = [
                [
                    pp.tile([P, B], dt, tag=f"a{li}_{i}", name=f"a{li}_{i}")
                    for i in range(n)
                ]
                for li, n in [(1, H1 // P), (2, H2 // P), (3, D1 // P)]
            ]
            acts.append([pp.tile([P, B], odt, tag="ot", name="ot")])

            # ---- PE warm-up: a couple of matmuls on a scratch tile keep
            # the PE busy (opening the HAM clock window) while the first
            # x / w1 DMA chunks are still in flight.  Only one column is
            # memset — just enough to materialize the tile — so the
            # first warm-up LDWEIGHTS issues as early as possible; the
            # garbage operand values never leave the scratch PSUM bank.
            if warm > 0:
                wdt = f32 if dt == mybir.dt.float32r else dt
                wsb = pp.tile([P, BT], wdt, tag="wsb", name="wsb")
                nc.vector.memset(wsb[:, 0:1], 0.0)
                wps = psp.tile([P, BT], f32, tag="ps", name="wps")
                lhs = wsb[:, 0:P]
                # N=256 warm-ups (213ns cold apiece) quantize the bridge
                # to first-data twice as finely as N=512 ones would
                rhs = wsb[:, 0:BT1]
                if dt == mybir.dt.float32r:
                    lhs = lhs.bitcast(dt)
                    rhs = rhs.bitcast(dt)
                for _ in range(warm):
                    nc.tensor.matmul(
                        wps[:, 0:BT1], lhs, rhs, start=True, stop=True
                    )

            # ---- input DMAs, issued in compute-need order on the two
            # HWDGE queues.  x is host-packed bt-major (col = bt*2*BT +
            # k*BT + b) so each transfer is contiguous.  sync carries x
            # (first-layer critical path), scalar carries the weights in
            # the order the wavefront consumes them — exactly two queues
            # active at the start so the first x / w1 chunks get the full
            # SDMA packet bandwidth.
            kx = S // P

            # The completion sem of a DMA fires ~1.5-2us after its data
            # lands (HBM write-receipt round trip under full 8-core
            # load), so first-matmul time is set by the size of the
            # first transfer; the warm-up matmuls above are sized to
            # bridge exactly that window.  x moves per 128KB L1
            # sub-tile, each carrying both k-halves under one sem.
            if add_bias:
                nc.scalar.dma_start(bti[:], b_d[:])
            nc.scalar.dma_start(wts[0][:, 0:512], w_ds[0][:, 0:512])
            for t in range(NT1):
                tsl = slice(t * kx * BT1, (t + 1) * kx * BT1)
                nc.sync.dma_start(xt[:, tsl], x_d[:, tsl])
            nc.scalar.dma_start(wts[0][:, 512:1024], w_ds[0][:, 512:1024])
            nc.scalar.dma_start(wts[1][:], w_ds[1][:])
            nc.scalar.dma_start(wts[3][:], w_ds[3][:])
            nc.scalar.dma_start(wts[2][:], w_ds[2][:])
            if add_bias:
                # advance ACT/DVE engine clocks past the bias DMA so the
                # real post-matmul ops carry a single (PE) wait each — the
                # AC/DVE instruction structs have one wait slot.
                nc.scalar.copy(scr[:, 0:1], bti[:, 0:1])
                nc.vector.tensor_copy(scr[:, 1:2], bti[:, 0:1])

            # ---- the 4-layer chain, emitted as a (bt + 2*layer) diagonal
            # wavefront: the PE's in-order queue then always has ready
            # later-layer work to chew while L1 waits on x DMAs.
            sched = sorted(
                ((bt + 2 * li, -li, bt) for li in range(4) for bt in range(NBT))
            )
            for _, nli, bt in sched:
                li = -nli
                kc, mc, boff, relu = _LAYERS[li]
                wt = wts[li]
                dsts = acts[li]
                srcs = acts[li - 1] if li > 0 else None
                if li == 0:
                    # layer 1 runs on 256-col sub-tiles, k-outer: each
                    # sub-tile's matmuls wait only on one 128KB x chunk
                    # (covering both k-halves), whose completion sem
                    # fires ~1.5us earlier than a 256KB transfer's.
                    for t in (2 * bt, 2 * bt + 1):
                        if warm > 0 and 1 <= t <= 3:
                            # filler scratch matmul: absorbs x-sem
                            # arrival jitter during the HAM cold window
                            # so a late chunk never idles the PE long
                            # enough to reset the clock-warming counter
                            nc.tensor.matmul(
                                wps[:, 0:BT1],
                                lhs,
                                rhs[:, 0:BT1],
                                start=True,
                                stop=True,
                            )
                        pss = [
                            psp.tile(
                                [P, BT1], f32, tag="ps", name=f"ps_l0_{t}_{m}"
                            )
                            for m in range(mc)
                        ]
                        for k in range(kc):
                            rhs = xt[
                                :, (t * kx + k) * BT1 : (t * kx + k + 1) * BT1
                            ]
                            for m in range(mc):
                                nc.tensor.matmul(
                                    pss[m][:],
                                    wt[:, (k * mc + m) * P : (k * mc + m + 1) * P],
                                    rhs,
                                    start=(k == 0),
                                    stop=(k == kc - 1),
                                )
                        for m in range(mc):
                            dst = dsts[m][:, t * BT1 : (t + 1) * BT1]
                            ps = pss[m]
                            if add_bias:
                                bias_ap = bti[:, boff + m : boff + m + 1]
                                if m < mc // 2:
                                    nc.scalar.activation(
                                        dst,
                                        ps[:],
                                        mybir.ActivationFunctionType.Relu,
                                        bias=bias_ap,
                                    )
                                else:
                                    nc.vector.tensor_scalar(
                                        dst,
                                        ps[:],
                                        bias_ap,
                                        0.0,
                                        mybir.AluOpType.add,
                                        mybir.AluOpType.max,
                                    )
                            elif m < mc // 2:
                                nc.scalar.activation(
                                    dst, ps[:], mybir.ActivationFunctionType.Relu
                                )
                            else:
                                nc.vector.tensor_scalar_max(dst, ps[:], 0.0)
                    continue
                if li == 3:
                    # final layer: the last batch-tile tapers down
                    # (256/128/128-col accumulations) so earlier chunks'
                    # copies and out-DMAs overlap later chunks' matmuls
                    # and the end-gating transfer is tiny.
                    ot = dsts[0]
                    widths = (256, 128, 128) if bt == NBT - 1 else (None,)
                    off = bt * BT
                    for hh, w in enumerate(widths):
                        if w is None:
                            hh, w = None, BT
                        ps = psp.tile([P, w], f32, tag="ps", name="ps")
                        for k in range(kc):
                            nc.tensor.matmul(
                                ps[:],
                                wt[:, k * P : (k + 1) * P],
                                srcs[k][:, off : off + w],
                                start=(k == 0),
                                stop=(k == kc - 1),
                            )
                        # PSUM -> SBUF split across ScalarE and VectorE
                        h = w // 2
                        dst = ot[:, off : off + w]
                        if add_bias:
                            bias_ap = bti[:, boff : boff + 1]
                            nc.scalar.activation(
                                dst[:, 0:h],
                                ps[:, 0:h],
                                mybir.ActivationFunctionType.Identity,
                                bias=bias_ap,
                            )
                            nc.vector.tensor_scalar_add(
                                dst[:, h:], ps[:, h:], bias_ap
                            )
                        else:
                            nc.scalar.activation(
                                dst[:, 0:h],
                                ps[:, 0:h],
                                mybir.ActivationFunctionType.Copy,
                            )
                            nc.vector.tensor_copy(dst[:, h:], ps[:, h:])
                        if hh is None:
                            eng = nc.sync if bt % 2 == 0 else nc.scalar
                        else:
                            eng = nc.sync if hh % 2 == 0 else nc.scalar
                        eng.dma_start(out_d[:, off : off + w], dst)
                        off += w
                    continue
                for m in range(mc):
                    # fixed engine per dst tile: one writer per tile
                    use_act = m < mc // 2 or mc == 1
                    ps = psp.tile([P, BT], f32, tag="ps", name="ps")
                    for k in range(kc):
                        rhs = srcs[k][:, bt * BT : (bt + 1) * BT]
                        nc.tensor.matmul(
                            ps[:],
                            wt[:, (k * mc + m) * P : (k * mc + m + 1) * P],
                            rhs,
                            start=(k == 0),
                            stop=(k == kc - 1),
                        )
                    dst = dsts[m][:, bt * BT : (bt + 1) * BT]
                    if add_bias:
                        bias_ap = bti[:, boff + m : boff + m + 1]
                        if use_act:
                            nc.scalar.activation(
                                dst,
                                ps[:],
                                mybir.ActivationFunctionType.Relu,
                                bias=bias_ap,
                            )
                        else:
                            nc.vector.tensor_scalar(
                                dst,
                                ps[:],
                                bias_ap,
                                0.0,
                                mybir.AluOpType.add,
                                mybir.AluOpType.max,
                            )
                    elif use_act:
                        nc.scalar.activation(
                            dst, ps[:], mybir.ActivationFunctionType.Relu
                        )
                    else:
                        nc.vector.tensor_scalar_max(dst, ps[:], 0.0)
    nc.compile()
    return nc


_BUILT: dict[tuple, bass.Bass] = {}


def _cfg():
    dt_name = os.environ.get("MADPS_DT", "bf16")
    warm = int(os.environ.get("MADPS_WARM", "13"))
    return dt_name, warm


def _get_nc(dt_name: str, add_bias: bool, warm: int) -> bass.Bass:
    key = (dt_name, add_bias, warm)
    if key not in _BUILT:
        _BUILT[key] = _build(dt_name, add_bias, warm)
    return _BUILT[key]


def _np_dt(dt_name: str):
    if dt_name == "bf16":
        import ml_dtypes

        return ml_dtypes.bfloat16
    return np.float32


def _packw(w: np.ndarray, np_dt) -> np.ndarray:
    """[K, M] -> [128, (K/128)*M], k-chunk-major: col (k*mc + m)*128 + j."""
    k, m = w.shape
    kc = k // P
    return np.ascontiguousarray(
        w.reshape(kc, P, m).transpose(1, 0, 2).reshape(P, -1).astype(np_dt)
    )


def _prepare(inputs, dt_name):
    """Returns (add_bias, in_maps) for run_bass_kernel_spmd."""
    np_dt = _np_dt(dt_name)

    x = np.asarray(inputs["inputs"], dtype=np.float32)
    sel_s = np.asarray(inputs["laac_shallow"]).reshape(-1).astype(np.int64)
    sel_d = np.asarray(inputs["laac_deep"]).reshape(-1).astype(np.int64)
    Ws1 = np.asarray(inputs["Ws1"], dtype=np.float32)
    Ws2 = np.asarray(inputs["Ws2"], dtype=np.float32)
    Wd1 = np.asarray(inputs["Wd1"], dtype=np.float32)
    Wd2 = np.asarray(inputs["Wd2"], dtype=np.float32)
    bs1 = np.asarray(inputs["bs1"], dtype=np.float32)
    bs2 = np.asarray(inputs["bs2"], dtype=np.float32)
    bd1 = np.asarray(inputs["bd1"], dtype=np.float32)
    bd2 = np.asarray(inputs["bd2"], dtype=np.float32)

    add_bias = any(
        float(np.abs(b).max()) != 0.0 for b in (bs1, bs2, bd1, bd2)
    )

    in_maps = []
    for a in range(A):
        es, ed = int(sel_s[a]), int(sel_d[a])
        # bt-major packing: col = bt*(S//P)*BT + k*BT + b
        xp = np.ascontiguousarray(
            x[a]
            .reshape(NT1, BT1, S // P, P)
            .transpose(3, 0, 2, 1)
            .reshape(P, -1)
            .astype(np_dt)
        )
        m = {
            "x": xp,
            "w1": _packw(Ws1[es], np_dt),
            "w2": _packw(Ws2[es], np_dt),
            "w3": _packw(Wd1[ed], np_dt),
            "w4": _packw(Wd2[ed], np_dt),
        }
        if add_bias:
            bias_cols = np.concatenate([bs1[es], bs2[es], bd1[ed], bd2[ed]])
            m["bias"] = np.ascontiguousarray(
                bias_cols.reshape(11, P).T, dtype=np.float32
            )
        in_maps.append(m)
    return add_bias, in_maps


def kernel(**inputs) -> np.ndarray:
    dt_name, warm = _cfg()
    add_bias, in_maps = _prepare(inputs, dt_name)
    nc = _get_nc(dt_name, add_bias, warm)
    res = run_bass_kernel_spmd(nc, in_maps, list(range(A)))
    out = np.stack([np.asarray(res.results[a]["out"]).T for a in range(A)])
    return np.ascontiguousarray(out.astype(np.float32))


# revision 42
# speedup vs baseline: 1.0560x; 1.0560x over previous
"""MADPSNet MoE-routing kernel for 8 Trainium2 NeuronCores.

The reference computes every expert on the full stacked input and then
gathers one expert per agent.  The routing indices (laac_shallow /
laac_deep) are host-visible numpy values, so we do the routing on the
host: per agent we select the 4 weight matrices of its chosen experts
and run only the selected chain

    x[2048,256] @ W1[256,512] -> relu -> @ W2[512,256] -> relu
                -> @ W3[256,512] -> relu -> @ W4[512,128] (+bias)

One agent per NeuronCore (A == 8 == n_cores), no collectives.

Layout: everything feature-major on chip (features on the 128
partitions, batch on the free dim).  The host pre-packs

    x   [128, 4096]     col = t*512 + k*256 + b  (t = 256-col L1 sub-tile)
    wN  [128, K/128*M]  col = (k*mc + m)*128 + j   (k-chunk-major)
    bias[128, 11]       col j = 128-chunk j of [b1(4) b2(2) b3(4) b4(1)]

Data is bf16 (PSUM accumulates fp32; rel err ~5e-3, well inside the
2e-2 gate) so every input transfer is half the bytes of f32 and
LDWEIGHTS runs under FWL.

The schedule is built around a measured constraint: a DMA's completion
sem fires ~1.5-2us after its data lands (HBM write-receipt round trip
under full 8-core load), so the earliest-possible first real matmul is
set by the size of the first transfer, not by bandwidth.  Layer 1
therefore runs on 256-col sub-tiles whose 128KB x chunks (each
carrying both k-halves, so the k-sweep never waits mid-accumulation)
sem ~1.5us earlier than a 256KB tile would; x goes on the sync HWDGE
queue, all weights on the scalar HWDGE queue in consumption order.
Warm-up matmuls on a scratch tile (only one column memset, just enough
to materialize it) bridge from engine start (~7us) to first-data
(~10us), opening the HAM clock window so the stream runs at 2.4 GHz,
and filler scratch matmuls between the first layer-1 sub-tiles absorb
x-sem arrival jitter so a late chunk can't idle the PE long enough to
re-throttle the clock; layers are emitted as a (bt + 2*layer) diagonal
wavefront so the in-order PE queue always has ready work (measured PE
gaps < 1us total).  Post-matmul relu is split across ScalarE and VectorE with a
fixed engine per destination tile; the final layer's PSUM->SBUF copies
split across both engines, its out-DMAs alternate queues, and the last
batch-tile runs as two 256-col accumulations so the drain is one
half-size transfer + receipt deep.  The kernel returns out^T
[128, 2048] per core; the host transposes back.
"""

import os

import numpy as np

import concourse.bass as bass
import concourse.mybir as mybir
from concourse import bacc
from concourse.bass_utils import run_bass_kernel_spmd
from concourse.tile import TileContext

A, B, S = 8, 2048, 256
H1, H2, D1, D2 = 512, 256, 512, 128
P = 128
BT = 512            # batch tile (psum bank: 512 fp32) for layers 2-4
NBT = B // BT
BT1 = 256           # layer-1 sub-tile: its 128KB x chunk (both k-halves)
NT1 = B // BT1      # sems ~1.5us earlier than a 256KB transfer would

_DT_MAP = {
    "f32": mybir.dt.float32,
    "f32r": mybir.dt.float32r,
    "bf16": mybir.dt.bfloat16,
}

# layer: (k_chunks, m_chunks, bias col offset, relu?)
_LAYERS = [
    (S // P, H1 // P, 0, True),    # L1: 256 -> 512
    (H1 // P, H2 // P, 4, True),   # L2: 512 -> 256
    (H2 // P, D1 // P, 6, True),   # L3: 256 -> 512
    (D1 // P, D2 // P, 10, False), # L4: 512 -> 128
]


def _build(dt_name: str, add_bias: bool, warm: int) -> bass.Bass:
    dt = _DT_MAP[dt_name]
    f32 = mybir.dt.float32
    nc = bacc.Bacc(None, target_bir_lowering=False, debug=False)

    x_d = nc.dram_tensor("x", [P, (S // P) * B], dt, kind="ExternalInput")
    w_ds = [
        nc.dram_tensor("w1", [P, (S // P) * H1], dt, kind="ExternalInput"),
        nc.dram_tensor("w2", [P, (H1 // P) * H2], dt, kind="ExternalInput"),
        nc.dram_tensor("w3", [P, (H2 // P) * D1], dt, kind="ExternalInput"),
        nc.dram_tensor("w4", [P, (D1 // P) * D2], dt, kind="ExternalInput"),
    ]
    b_d = (
        nc.dram_tensor("bias", [P, 11], f32, kind="ExternalInput")
        if add_bias
        else None
    )
    # bf16 mode also stages + ships the output in bf16 (half the final
    # drain bytes; the host upcasts, adding ~2e-3 rel err — far inside
    # the 2e-2 gate)
    odt = dt if dt == mybir.dt.bfloat16 else f32
    out_d = nc.dram_tensor("out", [D2, B], odt, kind="ExternalOutput")

    with TileContext(nc) as tc:
        with (
            tc.tile_pool(name="persist", bufs=1) as pp,
            tc.tile_pool(name="psum", bufs=8, space="PSUM") as psp,
        ):
            xt = pp.tile([P, (S // P) * B], dt, tag="xt", name="xt")
            wts = [
                pp.tile(
                    [P, w_ds[i].shape[1]], dt, tag=f"w{i}", name=f"w{i}_sb"
                )
                for i in range(4)
            ]
            bti = (
                pp.tile([P, 11], f32, tag="bias", name="bias_sb")
                if add_bias
                else None
            )
            scr = (
                pp.tile([P, 2], f32, tag="scr", name="scr") if add_bias else None
            )
            acts = [
                [
                    pp.tile([P, B], dt, tag=f"a{li}_{i}", name=f"a{li}_{i}")
                    for i in range(n)
                ]
                for li, n in [(1, H1 // P), (2, H2 // P), (3, D1 // P)]
            ]
            acts.append([pp.tile([P, B], odt, tag="ot", name="ot")])

            # ---- PE warm-up: a couple of matmuls on a scratch tile keep
            # the PE busy (opening the HAM clock window) while the first
            # x / w1 DMA chunks are still in flight.  Only one column is
            # memset — just enough to materialize the tile — so the
            # first warm-up LDWEIGHTS issues as early as possible; the
            # garbage operand values never leave the scratch PSUM bank.
            if warm > 0:
                wdt = f32 if dt == mybir.dt.float32r else dt
                wsb = pp.tile([P, BT], wdt, tag="wsb", name="wsb")
                nc.vector.memset(wsb[:, 0:1], 0.0)
                wps = psp.tile([P, BT], f32, tag="ps", name="wps")
                lhs = wsb[:, 0:P]
                # N=256 warm-ups (213ns cold apiece) quantize the bridge
                # to first-data twice as finely as N=512 ones would
                rhs = wsb[:, 0:BT1]
                if dt == mybir.dt.float32r:
                    lhs = lhs.bitcast(dt)
                    rhs = rhs.bitcast(dt)
                for _ in range(warm):
                    nc.tensor.matmul(
                        wps[:, 0:BT1], lhs, rhs, start=True, stop=True
                    )

            # ---- input DMAs, issued in compute-need order on the two
            # HWDGE queues.  x is host-packed bt-major (col = bt*2*BT +
            # k*BT + b) so each transfer is contiguous.  sync carries x
            # (first-layer critical path), scalar carries the weights in
            # the order the wavefront consumes them — exactly two queues
            # active at the start so the first x / w1 chunks get the full
            # SDMA packet bandwidth.
            kx = S // P

            # The completion sem of a DMA fires ~1.5-2us after its data
            # lands (HBM write-receipt round trip under full 8-core
            # load), so first-matmul time is set by the size of the
            # first transfer; the warm-up matmuls above are sized to
            # bridge exactly that window.  x moves per 128KB L1
            # sub-tile, each carrying both k-halves under one sem.
            if add_bias:
                nc.scalar.dma_start(bti[:], b_d[:])
            nc.scalar.dma_start(wts[0][:, 0:512], w_ds[0][:, 0:512])
            for t in range(NT1):
                tsl = slice(t * kx * BT1, (t + 1) * kx * BT1)
                nc.sync.dma_start(xt[:, tsl], x_d[:, tsl])
            nc.scalar.dma_start(wts[0][:, 512:1024], w_ds[0][:, 512:1024])
            # w2 in k-halves: the k-outer L2 sweep only needs the first
            # half (k0+k1 blocks) to start, so a slow w2 can't stall it
            nc.scalar.dma_start(wts[1][:, 0:512], w_ds[1][:, 0:512])
            nc.scalar.dma_start(wts[1][:, 512:1024], w_ds[1][:, 512:1024])
            nc.scalar.dma_start(wts[3][:], w_ds[3][:])
            nc.scalar.dma_start(wts[2][:], w_ds[2][:])
            if add_bias:
                # advance ACT/DVE engine clocks past the bias DMA so the
                # real post-matmul ops carry a single (PE) wait each — the
                # AC/DVE instruction structs have one wait slot.
                nc.scalar.copy(scr[:, 0:1], bti[:, 0:1])
                nc.vector.tensor_copy(scr[:, 1:2], bti[:, 0:1])

            # ---- the 4-layer chain, emitted as a (bt + 2*layer) diagonal
            # wavefront: the PE's in-order queue then always has ready
            # later-layer work to chew while L1 waits on x DMAs.
            sched = sorted(
                ((bt + 2 * li, -li, bt) for li in range(4) for bt in range(NBT))
            )
            for _, nli, bt in sched:
                li = -nli
                kc, mc, boff, relu = _LAYERS[li]
                wt = wts[li]
                dsts = acts[li]
                srcs = acts[li - 1] if li > 0 else None
                if li == 0:
                    # layer 1 runs on 256-col sub-tiles, k-outer: each
                    # sub-tile's matmuls wait only on one 128KB x chunk
                    # (covering both k-halves), whose completion sem
                    # fires ~1.5us earlier than a 256KB transfer's.
                    for t in (2 * bt, 2 * bt + 1):
                        if warm > 0 and 1 <= t <= 3:
                            # filler scratch matmul: absorbs x-sem
                            # arrival jitter during the HAM cold window
                            # so a late chunk never idles the PE long
                            # enough to reset the clock-warming counter
                            nc.tensor.matmul(
                                wps[:, 0:BT1],
                                lhs,
                                rhs[:, 0:BT1],
                                start=True,
                                stop=True,
                            )
                        pss = [
                            psp.tile(
                                [P, BT1], f32, tag="ps", name=f"ps_l0_{t}_{m}"
                            )
                            for m in range(mc)
                        ]
                        for k in range(kc):
                            rhs = xt[
                                :, (t * kx + k) * BT1 : (t * kx + k + 1) * BT1
                            ]
                            for m in range(mc):
                                nc.tensor.matmul(
                                    pss[m][:],
                                    wt[:, (k * mc + m) * P : (k * mc + m + 1) * P],
                                    rhs,
                                    start=(k == 0),
                                    stop=(k == kc - 1),
                                )
                        for m in range(mc):
                            dst = dsts[m][:, t * BT1 : (t + 1) * BT1]
                            ps = pss[m]
                            if add_bias:
                                bias_ap = bti[:, boff + m : boff + m + 1]
                                if m < mc // 2:
                                    nc.scalar.activation(
                                        dst,
                                        ps[:],
                                        mybir.ActivationFunctionType.Relu,
                                        bias=bias_ap,
                                    )
                                else:
                                    nc.vector.tensor_scalar(
                                        dst,
                                        ps[:],
                                        bias_ap,
                                        0.0,
                                        mybir.AluOpType.add,
                                        mybir.AluOpType.max,
                                    )
                            elif m < mc // 2:
                                nc.scalar.activation(
                                    dst, ps[:], mybir.ActivationFunctionType.Relu
                                )
                            else:
                                nc.vector.tensor_scalar_max(dst, ps[:], 0.0)
                    continue
                if li == 3:
                    # final layer: the last batch-tile tapers down
                    # (256/128/128-col accumulations) so earlier chunks'
                    # copies and out-DMAs overlap later chunks' matmuls
                    # and the end-gating transfer is tiny.
                    ot = dsts[0]
                    widths = (256, 128, 128) if bt == NBT - 1 else (None,)
                    off = bt * BT
                    for hh, w in enumerate(widths):
                        if w is None:
                            hh, w = None, BT
                        ps = psp.tile([P, w], f32, tag="ps", name="ps")
                        for k in range(kc):
                            nc.tensor.matmul(
                                ps[:],
                                wt[:, k * P : (k + 1) * P],
                                srcs[k][:, off : off + w],
                                start=(k == 0),
                                stop=(k == kc - 1),
                            )
                        # PSUM -> SBUF split across ScalarE and VectorE
                        h = w // 2
                        dst = ot[:, off : off + w]
                        if add_bias:
                            bias_ap = bti[:, boff : boff + 1]
                            nc.scalar.activation(
                                dst[:, 0:h],
                                ps[:, 0:h],
                                mybir.ActivationFunctionType.Identity,
                                bias=bias_ap,
                            )
                            nc.vector.tensor_scalar_add(
                                dst[:, h:], ps[:, h:], bias_ap
                            )
                        else:
                            nc.scalar.activation(
                                dst[:, 0:h],
                                ps[:, 0:h],
                                mybir.ActivationFunctionType.Copy,
                            )
                            nc.vector.tensor_copy(dst[:, h:], ps[:, h:])
                        if hh is None:
                            eng = nc.sync if bt % 2 == 0 else nc.scalar
                        else:
                            eng = nc.sync if hh % 2 == 0 else nc.scalar
                        eng.dma_start(out_d[:, off : off + w], dst)
                        off += w
                    continue
                if li == 1:
                    # L2 runs k-outer so its first matmuls need only the
                    # first k-block of w2, not the whole tensor
                    pss = [
                        psp.tile([P, BT], f32, tag="ps", name=f"ps_l1_{bt}_{m}")
                        for m in range(mc)
                    ]
                    for k in range(kc):
                        rhs = srcs[k][:, bt * BT : (bt + 1) * BT]
                        for m in range(mc):
                            nc.tensor.matmul(
                                pss[m][:],
                                wt[:, (k * mc + m) * P : (k * mc + m + 1) * P],
                                rhs,
                                start=(k == 0),
                                stop=(k == kc - 1),
                            )
                for m in range(mc):
                    # fixed engine per dst tile: one writer per tile
                    use_act = m < mc // 2 or mc == 1
                    if li == 1:
                        ps = pss[m]
                    else:
                        ps = psp.tile([P, BT], f32, tag="ps", name="ps")
                        for k in range(kc):
                            rhs = srcs[k][:, bt * BT : (bt + 1) * BT]
                            nc.tensor.matmul(
                                ps[:],
                                wt[:, (k * mc + m) * P : (k * mc + m + 1) * P],
                                rhs,
                                start=(k == 0),
                                stop=(k == kc - 1),
                            )
                    dst = dsts[m][:, bt * BT : (bt + 1) * BT]
                    if add_bias:
                        bias_ap = bti[:, boff + m : boff + m + 1]
                        if use_act:
                            nc.scalar.activation(
                                dst,
                                ps[:],
                                mybir.ActivationFunctionType.Relu,
                                bias=bias_ap,
                            )
                        else:
                            nc.vector.tensor_scalar(
                                dst,
                                ps[:],
                                bias_ap,
                                0.0,
                                mybir.AluOpType.add,
                                mybir.AluOpType.max,
                            )
                    elif use_act:
                        nc.scalar.activation(
                            dst, ps[:], mybir.ActivationFunctionType.Relu
                        )
                    else:
                        nc.vector.tensor_scalar_max(dst, ps[:], 0.0)
    nc.compile()
    return nc


_BUILT: dict[tuple, bass.Bass] = {}


def _cfg():
    dt_name = os.environ.get("MADPS_DT", "bf16")
    warm = int(os.environ.get("MADPS_WARM", "13"))
    return dt_name, warm


def _get_nc(dt_name: str, add_bias: bool, warm: int) -> bass.Bass:
    key = (dt_name, add_bias, warm)
    if key not in _BUILT:
        _BUILT[key] = _build(dt_name, add_bias, warm)
    return _BUILT[key]


def _np_dt(dt_name: str):
    if dt_name == "bf16":
        import ml_dtypes

        return ml_dtypes.bfloat16
    return np.float32


def _packw(w: np.ndarray, np_dt) -> np.ndarray:
    """[K, M] -> [128, (K/128)*M], k-chunk-major: col (k*mc + m)*128 + j."""
    k, m = w.shape
    kc = k // P
    return np.ascontiguousarray(
        w.reshape(kc, P, m).transpose(1, 0, 2).reshape(P, -1).astype(np_dt)
    )


def _prepare(inputs, dt_name):
    """Returns (add_bias, in_maps) for run_bass_kernel_spmd."""
    np_dt = _np_dt(dt_name)

    x = np.asarray(inputs["inputs"], dtype=np.float32)
    sel_s = np.asarray(inputs["laac_shallow"]).reshape(-1).astype(np.int64)
    sel_d = np.asarray(inputs["laac_deep"]).reshape(-1).astype(np.int64)
    Ws1 = np.asarray(inputs["Ws1"], dtype=np.float32)
    Ws2 = np.asarray(inputs["Ws2"], dtype=np.float32)
    Wd1 = np.asarray(inputs["Wd1"], dtype=np.float32)
    Wd2 = np.asarray(inputs["Wd2"], dtype=np.float32)
    bs1 = np.asarray(inputs["bs1"], dtype=np.float32)
    bs2 = np.asarray(inputs["bs2"], dtype=np.float32)
    bd1 = np.asarray(inputs["bd1"], dtype=np.float32)
    bd2 = np.asarray(inputs["bd2"], dtype=np.float32)

    add_bias = any(
        float(np.abs(b).max()) != 0.0 for b in (bs1, bs2, bd1, bd2)
    )

    in_maps = []
    for a in range(A):
        es, ed = int(sel_s[a]), int(sel_d[a])
        # bt-major packing: col = bt*(S//P)*BT + k*BT + b
        xp = np.ascontiguousarray(
            x[a]
            .reshape(NT1, BT1, S // P, P)
            .transpose(3, 0, 2, 1)
            .reshape(P, -1)
            .astype(np_dt)
        )
        m = {
            "x": xp,
            "w1": _packw(Ws1[es], np_dt),
            "w2": _packw(Ws2[es], np_dt),
            "w3": _packw(Wd1[ed], np_dt),
            "w4": _packw(Wd2[ed], np_dt),
        }
        if add_bias:
            bias_cols = np.concatenate([bs1[es], bs2[es], bd1[ed], bd2[ed]])
            m["bias"] = np.ascontiguousarray(
                bias_cols.reshape(11, P).T, dtype=np.float32
            )
        in_maps.append(m)
    return add_bias, in_maps


def kernel(**inputs) -> np.ndarray:
    dt_name, warm = _cfg()
    add_bias, in_maps = _prepare(inputs, dt_name)
    nc = _get_nc(dt_name, add_bias, warm)
    res = run_bass_kernel_spmd(nc, in_maps, list(range(A)))
    out = np.stack([np.asarray(res.results[a]["out"]).T for a in range(A)])
    return np.ascontiguousarray(out.astype(np.float32))
